# revision 12
# baseline (speedup 1.0000x reference)
"""BCE + weighted Dice loss on 8 Trainium2 NeuronCores.

Full inputs logits/targets [4,3,128,128,128] f32 are sharded along the depth
axis D=128 into 8 slices of 16 and converted to bf16 on the host (halves DMA;
targets are {0,1} so exact, logits rounding shifts the loss by ~1e-5 rel).
Each core reduces its shard to per-(b,c) partial sums; the host combines them.

Math notes (s := sigmoid(-x)):
  sigmoid(x)   = 1 - s
  softplus(x)  = -ln(s)
  sum(prob)    = N - sum(s)
  sum(prob*t)  = sum(t) - sum(s*t)
  bce_sum      = -sum(ln s) - sum(x*t)
  pred         = (x >= 0.5);  t*pred = t*(x>=0.5)

Work is organized in 3 "quads" of 4 (b,c) slabs, [128, 8192] tiles, so the
per-op fixed costs (ScalarE 352-cycle ramp, DVE drains, semaphores) amortize.
Global sums (sum s, sum ln s, sum x*t) accumulate per quad; per-(b,c) sums
(sum t, sum pred, sum t*pred) are produced per 2048-column slab slice.

Engine split:
  ScalarE: s = sigmoid(-x) (+accum), ln(s) (+accum), alternating per quad,
      chained via zero-bias tiles so the activation table set loads 6x total
  VectorE: pred = (x >= 0.5) (bf16 4x mode), sum(x*t) via fused
      scalar_tensor_tensor accumulate, PSUM diag-mask extractions
  TensorE: diagonal-trick matmuls for sum(s*t) (global) and sum(t*pred)
      (per slab); ones-matmuls for per-slab sum(t) / sum(pred)

The diagonal trick: accumulating chunk matmuls A[:,c128].T @ B[:,c128] into
one PSUM bank leaves sum_c sum_p A[p,cm]*B[p,cn] at [m,n]; the diagonal
m == n carries the elementwise dot product. Masking by the identity (a tiny
host-supplied input) recovers sum(A*B) without any slow DVE reduce.

Device outputs per core:
  stats_act [128, 6]: ScalarE accums per quad (sum s, sum ln s)
  stats_dve [128, 3]: VectorE accums per quad (sum x*t)
  diag_st [128, 128] f32: masked global-diag accumulator -> sum(s*t)
  diag_tp [12, 128, 128] f32: per-slab masked accumulators -> sum(t*pred)
  trows [96, 2048] f32: ones-matmul row banks; slab s row lives at
      partition (s%3)*32, cols 1024*q + 512*(s//6) + 256*((s//3)%2) for
      quantity q in {0: sum(t), 1: sum(pred)}, 256 wide
"""

import sys

if "/opt/trn_rl_repo" not in sys.path:
    sys.path.insert(0, "/opt/trn_rl_repo")

import numpy as np

import concourse.bacc as bacc
import concourse.mybir as mybir
from concourse import tile
from concourse.alu_op_type import AluOpType
from concourse.bass_utils import run_bass_kernel_spmd

# Problem geometry (hardcoded per harness contract).
B, C, D, H, W = 4, 3, 128, 128, 128
N_CORES = 8
D_SHARD = D // N_CORES            # 16
SLABS = B * C                     # 12 (b,c) slabs per core
P = 128                           # SBUF partitions
F = D_SHARD * H * W // P          # 2048 free elems per slab per partition
N_SLAB = P * F                    # 262144 elems per core-slab
N_TOTAL = B * C * D * H * W
QUADS = 3
QS = SLABS // QUADS               # 4 slabs per quad
QF = QS * F                       # 8192 free elems per quad tile

_CACHED = {}


def _build():
    if "nc" in _CACHED:
        return _CACHED["nc"]
    AFT = mybir.ActivationFunctionType
    f32 = mybir.dt.float32
    bf16 = mybir.dt.bfloat16

    nc = bacc.Bacc("TRN2", target_bir_lowering=False, debug=False,
                   num_devices=N_CORES)
    x_d = nc.dram_tensor("logits", [QUADS, P, QF], bf16, kind="ExternalInput")
    t_d = nc.dram_tensor("targets", [QUADS, P, QF], bf16, kind="ExternalInput")
    id_d = nc.dram_tensor("ident", [P, 128], bf16, kind="ExternalInput")
    sa_d = nc.dram_tensor("stats_act", [P, 2 * QUADS], f32, kind="ExternalOutput")
    sd_d = nc.dram_tensor("stats_dve", [P, QUADS], f32, kind="ExternalOutput")
    dst_d = nc.dram_tensor("diag_st", [P, 128], f32, kind="ExternalOutput")
    dtp_d = nc.dram_tensor("diag_tp", [SLABS, P, 128], f32, kind="ExternalOutput")
    tr_d = nc.dram_tensor("trows", [96, 2048], f32, kind="ExternalOutput")

    with tile.TileContext(nc) as tc:
        with (
            tc.tile_pool(name="xt", bufs=2) as xt_pool,
            tc.tile_pool(name="s", bufs=QUADS) as s_pool,
            tc.tile_pool(name="pred", bufs=2) as pred_pool,
            tc.tile_pool(name="scr", bufs=2) as scr_pool,
            tc.tile_pool(name="misc", bufs=1) as misc_pool,
            tc.tile_pool(name="psum", bufs=1, space="PSUM") as psum_pool,
        ):
            stats_act = misc_pool.tile([P, 2 * QUADS], f32)
            nc.vector.memset(stats_act[:], 0.0)
            stats_dve = misc_pool.tile([P, QUADS], f32)
            nc.vector.memset(stats_dve[:], 0.0)
            ones = misc_pool.tile([P, 1], bf16)
            nc.vector.memset(ones[:], 1.0)
            ident = misc_pool.tile([P, 128], bf16)
            nc.sync.dma_start(ident[:], id_d[:])

            # PSUM banks (7 of 8): global st diag, 2 rotating tp diags,
            # 2+2 row banks for sum(t)/sum(pred).
            p_st = psum_pool.tile([P, 128], f32, name="p_st", tag="p_st")
            p_tp = [psum_pool.tile([P, 128], f32, name=f"p_tp{i}", tag=f"p_tp{i}")
                    for i in range(2)]
            p_t = [psum_pool.tile([P, 512], f32, name=f"p_t{i}", tag=f"p_t{i}")
                   for i in range(2)]
            p_pr = [psum_pool.tile([P, 512], f32, name=f"p_pr{i}", tag=f"p_pr{i}")
                    for i in range(2)]

            for q in range(QUADS):
                xq = xt_pool.tile([P, QF], bf16, tag="x", name=f"xq{q}")
                tq = xt_pool.tile([P, QF], bf16, tag="t", name=f"tq{q}")
                # Split each 2MiB load into 4 column slices on separate
                # engine DGE queues so the transfers run in parallel and the
                # quad completes sooner (ScalarE's sigmoid is gated on it).
                qeng = [nc.sync, nc.gpsimd]
                for k in range(4):
                    sl = slice(k * (QF // 4), (k + 1) * (QF // 4))
                    qeng[k % 2].dma_start(xq[:, sl], x_d[q][:, sl])
                for k in range(4):
                    sl = slice(k * (QF // 4), (k + 1) * (QF // 4))
                    qeng[(k + 1) % 2].dma_start(tq[:, sl], t_d[q][:, sl])

                # s = sigmoid(-x) (+ accum sum s for the quad)
                sq = s_pool.tile([P, QF], bf16, tag="s", name=f"sq{q}")
                nc.scalar.activation(
                    sq[:], xq[:], AFT.Sigmoid, scale=-1.0,
                    accum_out=stats_act[:, 2 * q:2 * q + 1],
                )
                # pred = (x >= 0.5) in bf16 (4x mode)
                pq = pred_pool.tile([P, QF], bf16, tag="pred", name=f"pq{q}")
                nc.vector.tensor_scalar(
                    out=pq[:], in0=xq[:], scalar1=0.5, scalar2=None,
                    op0=AluOpType.is_ge,
                )
                # sum(x*t) for the quad via fused STT accumulate
                uq = scr_pool.tile([P, QF], bf16, tag="u", name=f"uq{q}")
                nc.vector.scalar_tensor_tensor(
                    out=uq[:], in0=xq[:], scalar=1.0, in1=tq[:],
                    op0=AluOpType.mult, op1=AluOpType.mult,
                    accum_out=stats_dve[:, q:q + 1],
                )

                # Per-slab PE reductions.
                for j in range(QS):
                    s_i = q * QS + j
                    base = j * F
                    first = s_i == 0
                    last = s_i == SLABS - 1
                    for c in range(16):
                        sl = slice(base + c * 128, base + (c + 1) * 128)
                        nc.tensor.matmul(p_st[:, :], sq[:, sl], tq[:, sl],
                                         start=(first and c == 0),
                                         stop=(last and c == 15))
                    tp_bank = p_tp[s_i % 2]
                    for c in range(16):
                        sl = slice(base + c * 128, base + (c + 1) * 128)
                        nc.tensor.matmul(tp_bank[:, :], pq[:, sl], tq[:, sl],
                                         start=(c == 0), stop=(c == 15))
                    mtp = scr_pool.tile([P, 128], f32, tag="mtp",
                                        name=f"mtp{s_i}")
                    nc.vector.tensor_tensor(out=mtp[:], in0=tp_bank[:, :],
                                            in1=ident[:], op=AluOpType.mult)
                    nc.gpsimd.dma_start(dtp_d[s_i], mtp[:])

                    row = (s_i % 3) * 32
                    colblk = ((s_i // 3) % 2) * 256
                    t_bank = p_t[s_i // 6]
                    pr_bank = p_pr[s_i // 6]
                    for c in range(8):
                        sl = slice(base + c * 256, base + (c + 1) * 256)
                        nc.tensor.matmul(
                            t_bank[row:row + 1, colblk:colblk + 256],
                            ones[:], tq[:, sl], start=(c == 0), stop=(c == 7))
                    for c in range(8):
                        sl = slice(base + c * 256, base + (c + 1) * 256)
                        nc.tensor.matmul(
                            pr_bank[row:row + 1, colblk:colblk + 256],
                            ones[:], pq[:, sl], start=(c == 0), stop=(c == 7))

                # ln(s) for this quad (+ accum)
                lq = scr_pool.tile([P, QF], bf16, tag="l", bufs=1, name=f"lq{q}")
                nc.scalar.activation(
                    lq[:], sq[:], AFT.Ln,
                    accum_out=stats_act[:, 2 * q + 1:2 * q + 2],
                )

            # ---- Epilogue ----
            mst = misc_pool.tile([P, 128], f32)
            nc.vector.tensor_tensor(out=mst[:], in0=p_st[:, :], in1=ident[:],
                                    op=AluOpType.mult)
            nc.sync.dma_start(dst_d[:], mst[:])

            trows = misc_pool.tile([96, 2048], f32)
            for i in range(2):
                nc.vector.tensor_copy(trows[0:96, 512 * i:512 * (i + 1)],
                                      p_t[i][0:96, :])
                nc.vector.tensor_copy(trows[0:96, 1024 + 512 * i:1024 + 512 * (i + 1)],
                                      p_pr[i][0:96, :])
            nc.sync.dma_start(tr_d[:], trows[:])
            nc.sync.dma_start(sa_d[:], stats_act[:])
            nc.sync.dma_start(sd_d[:], stats_dve[:])

    nc.compile()
    _CACHED["nc"] = nc
    return nc


def _to_bf16_bits(a: np.ndarray) -> np.ndarray:
    """f32 -> bf16 bits with round-to-nearest-even, returned as uint16."""
    u = np.ascontiguousarray(a, dtype=np.float32).view(np.uint32)
    rounded = ((u + 0x7FFF + ((u >> 16) & 1)) >> 16).astype(np.uint16)
    return rounded


def _shard_inputs(logits: np.ndarray, targets: np.ndarray):
    import ml_dtypes

    bf = ml_dtypes.bfloat16
    xb = _to_bf16_bits(logits).view(bf)
    tb = _to_bf16_bits(targets).view(bf)
    eye = np.eye(P, 128, dtype=np.float32).astype(bf)
    in_maps = []
    for i in range(N_CORES):
        sl = slice(i * D_SHARD, (i + 1) * D_SHARD)
        x = np.ascontiguousarray(xb[:, :, sl]).reshape(QUADS, P, QF)
        t = np.ascontiguousarray(tb[:, :, sl]).reshape(QUADS, P, QF)
        in_maps.append({"logits": x, "targets": t, "ident": eye})
    return in_maps


def _combine(results):
    """Host-side reduction of per-core partials to the scalar loss."""
    EPS = 1e-9
    S_tp = np.zeros(SLABS)
    S_t = np.zeros(SLABS)
    S_pred = np.zeros(SLABS)
    S_s = 0.0
    S_l = 0.0
    S_xt = 0.0
    S_st = 0.0
    for r in results:
        sa = r["stats_act"].astype(np.float64)
        S_s += sa[:, 0::2].sum()
        S_l += sa[:, 1::2].sum()
        S_xt += r["stats_dve"].astype(np.float64).sum()
        S_st += r["diag_st"].astype(np.float64).sum()
        tr = r["trows"].astype(np.float64)
        dtp = r["diag_tp"].astype(np.float64)
        for s_i in range(SLABS):
            S_tp[s_i] += dtp[s_i].sum()
            row = (s_i % 3) * 32
            col = 512 * (s_i // 6) + 256 * ((s_i // 3) % 2)
            S_t[s_i] += tr[row, col:col + 256].sum()
            S_pred[s_i] += tr[row, 1024 + col:1024 + col + 256].sum()

    sum_prob = N_TOTAL - S_s
    sum_pt = S_t.sum() - S_st               # sum(prob * t)
    sum_sp = -S_l                           # sum(softplus(x))
    bce = (sum_sp - S_xt) / N_TOTAL

    union = sum_prob + S_t.sum()
    inter = 2.0 * sum_pt
    dice_loss = 1.0 - (inter + EPS) / union

    score = np.where(
        (S_t == 0) & (S_pred == 0),
        np.ones_like(S_t),
        (2.0 * S_tp + EPS) / (S_t + S_pred),
    ).reshape(B, C)
    per_class = score.mean(axis=0)

    loss = (bce + dice_loss * 0.5 + per_class[0] * 0.2
            + per_class[1] * 0.1 + per_class[2] * 0.2)
    return np.float32(loss)


def kernel(logits: np.ndarray, targets: np.ndarray) -> np.ndarray:
    nc = _build()
    in_maps = _shard_inputs(np.asarray(logits), np.asarray(targets))
    res = run_bass_kernel_spmd(nc, in_maps, list(range(N_CORES)))
    return _combine(res.results)


# revision 13
# speedup vs baseline: 1.0314x; 1.0314x over previous
"""BCE + weighted Dice loss on 8 Trainium2 NeuronCores.

Full inputs logits/targets [4,3,128,128,128] f32 are sharded along the depth
axis D=128 into 8 slices of 16 and converted to bf16 on the host (halves DMA;
targets are {0,1} so exact, logits rounding shifts the loss by ~1e-5 rel).
Each core reduces its shard to per-(b,c) partial sums; the host combines them.

Math notes (s := sigmoid(-x)):
  sigmoid(x)   = 1 - s
  softplus(x)  = -ln(s)
  sum(prob)    = N - sum(s)
  sum(prob*t)  = sum(t) - sum(s*t)
  bce_sum      = -sum(ln s) - sum(x*t)
  pred         = (x >= 0.5);  t*pred = t*(x>=0.5)

Work is organized in 3 "quads" of 4 (b,c) slabs, [128, 8192] tiles, so the
per-op fixed costs (ScalarE 352-cycle ramp, DVE drains, semaphores) amortize.
Global sums (sum s, sum ln s, sum x*t) accumulate per quad; per-(b,c) sums
(sum t, sum pred, sum t*pred) are produced per 2048-column slab slice.

Engine split:
  ScalarE: s = sigmoid(-x) (+accum), ln(s) (+accum), alternating per quad,
      chained via zero-bias tiles so the activation table set loads 6x total
  VectorE: pred = (x >= 0.5) (bf16 4x mode), sum(x*t) via fused
      scalar_tensor_tensor accumulate, PSUM diag-mask extractions
  TensorE: diagonal-trick matmuls for sum(s*t) (global) and sum(t*pred)
      (per slab); ones-matmuls for per-slab sum(t) / sum(pred)

The diagonal trick: accumulating chunk matmuls A[:,c128].T @ B[:,c128] into
one PSUM bank leaves sum_c sum_p A[p,cm]*B[p,cn] at [m,n]; the diagonal
m == n carries the elementwise dot product. Masking by the identity (a tiny
host-supplied input) recovers sum(A*B) without any slow DVE reduce.

Device outputs per core:
  stats_act [128, 6]: ScalarE accums per quad (sum s, sum ln s)
  stats_dve [128, 3]: VectorE accums per quad (sum x*t)
  diag_st [128, 128] f32: masked global-diag accumulator -> sum(s*t)
  diag_tp [12, 128, 128] f32: per-slab masked accumulators -> sum(t*pred)
  trows [96, 2048] f32: ones-matmul row banks; slab s row lives at
      partition (s%3)*32, cols 1024*q + 512*(s//6) + 256*((s//3)%2) for
      quantity q in {0: sum(t), 1: sum(pred)}, 256 wide
"""

import sys

if "/opt/trn_rl_repo" not in sys.path:
    sys.path.insert(0, "/opt/trn_rl_repo")

import numpy as np

import concourse.bacc as bacc
import concourse.mybir as mybir
from concourse import tile
from concourse.alu_op_type import AluOpType
from concourse.bass_utils import run_bass_kernel_spmd

# Problem geometry (hardcoded per harness contract).
B, C, D, H, W = 4, 3, 128, 128, 128
N_CORES = 8
D_SHARD = D // N_CORES            # 16
SLABS = B * C                     # 12 (b,c) slabs per core
P = 128                           # SBUF partitions
F = D_SHARD * H * W // P          # 2048 free elems per slab per partition
N_SLAB = P * F                    # 262144 elems per core-slab
N_TOTAL = B * C * D * H * W
QUADS = 3
QS = SLABS // QUADS               # 4 slabs per quad
QF = QS * F                       # 8192 free elems per quad tile

_CACHED = {}


def _build():
    if "nc" in _CACHED:
        return _CACHED["nc"]
    AFT = mybir.ActivationFunctionType
    f32 = mybir.dt.float32
    bf16 = mybir.dt.bfloat16

    nc = bacc.Bacc("TRN2", target_bir_lowering=False, debug=False,
                   num_devices=N_CORES)
    x_d = nc.dram_tensor("logits", [QUADS, P, QF], bf16, kind="ExternalInput")
    t_d = nc.dram_tensor("targets", [QUADS, P, QF], bf16, kind="ExternalInput")
    id_d = nc.dram_tensor("ident", [P, 128], bf16, kind="ExternalInput")
    sa_d = nc.dram_tensor("stats_act", [P, 2 * QUADS], f32, kind="ExternalOutput")
    sd_d = nc.dram_tensor("stats_dve", [P, QUADS], f32, kind="ExternalOutput")
    dst_d = nc.dram_tensor("diag_st", [P, 128], f32, kind="ExternalOutput")
    dtp_d = nc.dram_tensor("diag_tp", [SLABS, P, 128], f32, kind="ExternalOutput")
    tr_d = nc.dram_tensor("trows", [96, 2048], f32, kind="ExternalOutput")

    with tile.TileContext(nc) as tc:
        with (
            tc.tile_pool(name="xt", bufs=2) as xt_pool,
            tc.tile_pool(name="s", bufs=QUADS) as s_pool,
            tc.tile_pool(name="pred", bufs=2) as pred_pool,
            tc.tile_pool(name="scr", bufs=2) as scr_pool,
            tc.tile_pool(name="misc", bufs=1) as misc_pool,
            tc.tile_pool(name="psum", bufs=1, space="PSUM") as psum_pool,
        ):
            stats_act = misc_pool.tile([P, 2 * QUADS], f32)
            nc.vector.memset(stats_act[:], 0.0)
            stats_dve = misc_pool.tile([P, QUADS], f32)
            nc.vector.memset(stats_dve[:], 0.0)
            ones = misc_pool.tile([P, 1], bf16)
            nc.vector.memset(ones[:], 1.0)
            ident = misc_pool.tile([P, 128], bf16)
            nc.sync.dma_start(ident[:], id_d[:])

            # PSUM banks (7 of 8): global st diag, 2 rotating tp diags,
            # 2+2 row banks for sum(t)/sum(pred).
            p_st = psum_pool.tile([P, 128], f32, name="p_st", tag="p_st")
            p_tp = [psum_pool.tile([P, 128], f32, name=f"p_tp{i}", tag=f"p_tp{i}")
                    for i in range(2)]
            p_t = [psum_pool.tile([P, 512], f32, name=f"p_t{i}", tag=f"p_t{i}")
                   for i in range(2)]
            p_pr = [psum_pool.tile([P, 512], f32, name=f"p_pr{i}", tag=f"p_pr{i}")
                    for i in range(2)]

            for q in range(QUADS):
                xq = xt_pool.tile([P, QF], bf16, tag="x", name=f"xq{q}")
                tq = xt_pool.tile([P, QF], bf16, tag="t", name=f"tq{q}")
                # Split each 2MiB load into 4 column slices on separate
                # engine DGE queues so the transfers run in parallel and the
                # quad completes sooner (ScalarE's sigmoid is gated on it).
                nc.sync.dma_start(xq[:], x_d[q])
                nc.sync.dma_start(tq[:], t_d[q])

                # s = sigmoid(-x) (+ accum sum s for the quad)
                sq = s_pool.tile([P, QF], bf16, tag="s", name=f"sq{q}")
                nc.scalar.activation(
                    sq[:], xq[:], AFT.Sigmoid, scale=-1.0,
                    accum_out=stats_act[:, 2 * q:2 * q + 1],
                )
                # pred = (x >= 0.5) in bf16 (4x mode)
                pq = pred_pool.tile([P, QF], bf16, tag="pred", name=f"pq{q}")
                nc.vector.tensor_scalar(
                    out=pq[:], in0=xq[:], scalar1=0.5, scalar2=None,
                    op0=AluOpType.is_ge,
                )
                # sum(x*t) for the quad via fused STT accumulate
                uq = scr_pool.tile([P, QF], bf16, tag="u", name=f"uq{q}")
                nc.vector.scalar_tensor_tensor(
                    out=uq[:], in0=xq[:], scalar=1.0, in1=tq[:],
                    op0=AluOpType.mult, op1=AluOpType.mult,
                    accum_out=stats_dve[:, q:q + 1],
                )

                # Per-slab PE reductions.
                for j in range(QS):
                    s_i = q * QS + j
                    base = j * F
                    first = s_i == 0
                    last = s_i == SLABS - 1
                    for c in range(16):
                        sl = slice(base + c * 128, base + (c + 1) * 128)
                        nc.tensor.matmul(p_st[:, :], sq[:, sl], tq[:, sl],
                                         start=(first and c == 0),
                                         stop=(last and c == 15))
                    tp_bank = p_tp[s_i % 2]
                    for c in range(16):
                        sl = slice(base + c * 128, base + (c + 1) * 128)
                        nc.tensor.matmul(tp_bank[:, :], pq[:, sl], tq[:, sl],
                                         start=(c == 0), stop=(c == 15))
                    mtp = scr_pool.tile([P, 128], f32, tag="mtp",
                                        name=f"mtp{s_i}")
                    nc.vector.tensor_tensor(out=mtp[:], in0=tp_bank[:, :],
                                            in1=ident[:], op=AluOpType.mult)
                    nc.gpsimd.dma_start(dtp_d[s_i], mtp[:])

                    row = (s_i % 3) * 32
                    colblk = ((s_i // 3) % 2) * 256
                    t_bank = p_t[s_i // 6]
                    pr_bank = p_pr[s_i // 6]
                    for c in range(8):
                        sl = slice(base + c * 256, base + (c + 1) * 256)
                        nc.tensor.matmul(
                            t_bank[row:row + 1, colblk:colblk + 256],
                            ones[:], tq[:, sl], start=(c == 0), stop=(c == 7))
                    for c in range(8):
                        sl = slice(base + c * 256, base + (c + 1) * 256)
                        nc.tensor.matmul(
                            pr_bank[row:row + 1, colblk:colblk + 256],
                            ones[:], pq[:, sl], start=(c == 0), stop=(c == 7))

                # ln(s) for this quad (+ accum)
                lq = scr_pool.tile([P, QF], bf16, tag="l", bufs=1, name=f"lq{q}")
                nc.scalar.activation(
                    lq[:], sq[:], AFT.Ln,
                    accum_out=stats_act[:, 2 * q + 1:2 * q + 2],
                )

            # ---- Epilogue ----
            mst = misc_pool.tile([P, 128], f32)
            nc.vector.tensor_tensor(out=mst[:], in0=p_st[:, :], in1=ident[:],
                                    op=AluOpType.mult)
            nc.sync.dma_start(dst_d[:], mst[:])

            trows = misc_pool.tile([96, 2048], f32)
            for i in range(2):
                nc.vector.tensor_copy(trows[0:96, 512 * i:512 * (i + 1)],
                                      p_t[i][0:96, :])
                nc.vector.tensor_copy(trows[0:96, 1024 + 512 * i:1024 + 512 * (i + 1)],
                                      p_pr[i][0:96, :])
            nc.sync.dma_start(tr_d[:], trows[:])
            nc.sync.dma_start(sa_d[:], stats_act[:])
            nc.sync.dma_start(sd_d[:], stats_dve[:])

    nc.compile()
    _CACHED["nc"] = nc
    return nc


def _to_bf16_bits(a: np.ndarray) -> np.ndarray:
    """f32 -> bf16 bits with round-to-nearest-even, returned as uint16."""
    u = np.ascontiguousarray(a, dtype=np.float32).view(np.uint32)
    rounded = ((u + 0x7FFF + ((u >> 16) & 1)) >> 16).astype(np.uint16)
    return rounded


def _shard_inputs(logits: np.ndarray, targets: np.ndarray):
    import ml_dtypes

    bf = ml_dtypes.bfloat16
    xb = _to_bf16_bits(logits).view(bf)
    tb = _to_bf16_bits(targets).view(bf)
    eye = np.eye(P, 128, dtype=np.float32).astype(bf)
    in_maps = []
    for i in range(N_CORES):
        sl = slice(i * D_SHARD, (i + 1) * D_SHARD)
        x = np.ascontiguousarray(xb[:, :, sl]).reshape(QUADS, P, QF)
        t = np.ascontiguousarray(tb[:, :, sl]).reshape(QUADS, P, QF)
        in_maps.append({"logits": x, "targets": t, "ident": eye})
    return in_maps


def _combine(results):
    """Host-side reduction of per-core partials to the scalar loss."""
    EPS = 1e-9
    S_tp = np.zeros(SLABS)
    S_t = np.zeros(SLABS)
    S_pred = np.zeros(SLABS)
    S_s = 0.0
    S_l = 0.0
    S_xt = 0.0
    S_st = 0.0
    for r in results:
        sa = r["stats_act"].astype(np.float64)
        S_s += sa[:, 0::2].sum()
        S_l += sa[:, 1::2].sum()
        S_xt += r["stats_dve"].astype(np.float64).sum()
        S_st += r["diag_st"].astype(np.float64).sum()
        tr = r["trows"].astype(np.float64)
        dtp = r["diag_tp"].astype(np.float64)
        for s_i in range(SLABS):
            S_tp[s_i] += dtp[s_i].sum()
            row = (s_i % 3) * 32
            col = 512 * (s_i // 6) + 256 * ((s_i // 3) % 2)
            S_t[s_i] += tr[row, col:col + 256].sum()
            S_pred[s_i] += tr[row, 1024 + col:1024 + col + 256].sum()

    sum_prob = N_TOTAL - S_s
    sum_pt = S_t.sum() - S_st               # sum(prob * t)
    sum_sp = -S_l                           # sum(softplus(x))
    bce = (sum_sp - S_xt) / N_TOTAL

    union = sum_prob + S_t.sum()
    inter = 2.0 * sum_pt
    dice_loss = 1.0 - (inter + EPS) / union

    score = np.where(
        (S_t == 0) & (S_pred == 0),
        np.ones_like(S_t),
        (2.0 * S_tp + EPS) / (S_t + S_pred),
    ).reshape(B, C)
    per_class = score.mean(axis=0)

    loss = (bce + dice_loss * 0.5 + per_class[0] * 0.2
            + per_class[1] * 0.1 + per_class[2] * 0.2)
    return np.float32(loss)


def kernel(logits: np.ndarray, targets: np.ndarray) -> np.ndarray:
    nc = _build()
    in_maps = _shard_inputs(np.asarray(logits), np.asarray(targets))
    res = run_bass_kernel_spmd(nc, in_maps, list(range(N_CORES)))
    return _combine(res.results)


# revision 14
# speedup vs baseline: 1.0358x; 1.0042x over previous
"""BCE + weighted Dice loss on 8 Trainium2 NeuronCores.

Full inputs logits/targets [4,3,128,128,128] f32 are sharded along the depth
axis D=128 into 8 slices of 16 and converted to bf16 on the host (halves DMA;
targets are {0,1} so exact, logits rounding shifts the loss by ~1e-5 rel).
Each core reduces its shard to per-(b,c) partial sums; the host combines them.

Math notes (s := sigmoid(-x)):
  sigmoid(x)   = 1 - s
  softplus(x)  = -ln(s)
  sum(prob)    = N - sum(s)
  sum(prob*t)  = sum(t) - sum(s*t)
  bce_sum      = -sum(ln s) - sum(x*t)
  pred         = (x >= 0.5);  t*pred = t*(x>=0.5)

Work is organized in 3 "quads" of 4 (b,c) slabs, [128, 8192] tiles, so the
per-op fixed costs (ScalarE 352-cycle ramp, DVE drains, semaphores) amortize.
Global sums (sum s, sum ln s, sum x*t) accumulate per quad; per-(b,c) sums
(sum t, sum pred, sum t*pred) are produced per 2048-column slab slice.

Engine split:
  ScalarE: s = sigmoid(-x) (+accum), ln(s) (+accum), alternating per quad,
      chained via zero-bias tiles so the activation table set loads 6x total
  VectorE: pred = (x >= 0.5) (bf16 4x mode), sum(x*t) via fused
      scalar_tensor_tensor accumulate, PSUM diag-mask extractions
  TensorE: diagonal-trick matmuls for sum(s*t) (global) and sum(t*pred)
      (per slab); ones-matmuls for per-slab sum(t) / sum(pred)

The diagonal trick: accumulating chunk matmuls A[:,c128].T @ B[:,c128] into
one PSUM bank leaves sum_c sum_p A[p,cm]*B[p,cn] at [m,n]; the diagonal
m == n carries the elementwise dot product. Masking by the identity (a tiny
host-supplied input) recovers sum(A*B) without any slow DVE reduce.

Device outputs per core:
  stats_act [128, 6]: ScalarE accums per quad (sum s, sum ln s)
  stats_dve [128, 3]: VectorE accums per quad (sum x*t)
  diag_st [128, 128] f32: masked global-diag accumulator -> sum(s*t)
  diag_tp [12, 128, 128] f32: per-slab masked accumulators -> sum(t*pred)
  trows [96, 2048] f32: ones-matmul row banks; slab s row lives at
      partition (s%3)*32, cols 1024*q + 512*(s//6) + 256*((s//3)%2) for
      quantity q in {0: sum(t), 1: sum(pred)}, 256 wide
"""

import sys

if "/opt/trn_rl_repo" not in sys.path:
    sys.path.insert(0, "/opt/trn_rl_repo")

import numpy as np

import concourse.bacc as bacc
import concourse.mybir as mybir
from concourse import tile
from concourse.alu_op_type import AluOpType
from concourse.bass_utils import run_bass_kernel_spmd

# Problem geometry (hardcoded per harness contract).
B, C, D, H, W = 4, 3, 128, 128, 128
N_CORES = 8
D_SHARD = D // N_CORES            # 16
SLABS = B * C                     # 12 (b,c) slabs per core
P = 128                           # SBUF partitions
F = D_SHARD * H * W // P          # 2048 free elems per slab per partition
N_SLAB = P * F                    # 262144 elems per core-slab
N_TOTAL = B * C * D * H * W
QUADS = 3
QS = SLABS // QUADS               # 4 slabs per quad
QF = QS * F                       # 8192 free elems per quad tile

_CACHED = {}


def _build():
    if "nc" in _CACHED:
        return _CACHED["nc"]
    AFT = mybir.ActivationFunctionType
    f32 = mybir.dt.float32
    bf16 = mybir.dt.bfloat16

    nc = bacc.Bacc("TRN2", target_bir_lowering=False, debug=False,
                   num_devices=N_CORES)
    x_d = nc.dram_tensor("logits", [QUADS, P, QF], bf16, kind="ExternalInput")
    t_d = nc.dram_tensor("targets", [QUADS, P, QF], bf16, kind="ExternalInput")
    id_d = nc.dram_tensor("ident", [P, 128], bf16, kind="ExternalInput")
    sa_d = nc.dram_tensor("stats_act", [P, 2 * QUADS], f32, kind="ExternalOutput")
    sd_d = nc.dram_tensor("stats_dve", [P, QUADS], f32, kind="ExternalOutput")
    dst_d = nc.dram_tensor("diag_st", [P, 128], f32, kind="ExternalOutput")
    dtp_d = nc.dram_tensor("diag_tp", [SLABS, P, 128], f32, kind="ExternalOutput")
    tr_d = nc.dram_tensor("trows", [96, 2048], f32, kind="ExternalOutput")

    with tile.TileContext(nc) as tc:
        with (
            tc.tile_pool(name="xt", bufs=2) as xt_pool,
            tc.tile_pool(name="s", bufs=QUADS) as s_pool,
            tc.tile_pool(name="pred", bufs=2) as pred_pool,
            tc.tile_pool(name="scr", bufs=2) as scr_pool,
            tc.tile_pool(name="misc", bufs=1) as misc_pool,
            tc.tile_pool(name="psum", bufs=1, space="PSUM") as psum_pool,
        ):
            stats_act = misc_pool.tile([P, 2 * QUADS], f32)
            nc.vector.memset(stats_act[:], 0.0)
            stats_dve = misc_pool.tile([P, QUADS], f32)
            nc.vector.memset(stats_dve[:], 0.0)
            ones = misc_pool.tile([P, 1], bf16)
            nc.vector.memset(ones[:], 1.0)
            ident = misc_pool.tile([P, 128], bf16)
            nc.sync.dma_start(ident[:], id_d[:])

            # PSUM banks (7 of 8): global st diag, 2 rotating tp diags,
            # 2+2 row banks for sum(t)/sum(pred).
            p_st = psum_pool.tile([P, 128], f32, name="p_st", tag="p_st")
            p_tp = [psum_pool.tile([P, 128], f32, name=f"p_tp{i}", tag=f"p_tp{i}")
                    for i in range(2)]
            p_t = [psum_pool.tile([P, 512], f32, name=f"p_t{i}", tag=f"p_t{i}")
                   for i in range(2)]
            p_pr = [psum_pool.tile([P, 512], f32, name=f"p_pr{i}", tag=f"p_pr{i}")
                    for i in range(2)]

            for q in range(QUADS):
                xq = xt_pool.tile([P, QF], bf16, tag="x", name=f"xq{q}")
                tq = xt_pool.tile([P, QF], bf16, tag="t", name=f"tq{q}")
                if q == 0:
                    # Four slices enqueued before any other transfer: the DMA
                    # engines fair-share across outstanding transfers, so one
                    # monolithic 2MiB first load completes far too late and
                    # stalls the ScalarE sigmoid chain at the very start.
                    for k in range(4):
                        sl = slice(k * F, (k + 1) * F)
                        nc.sync.dma_start(xq[:, sl], x_d[q][:, sl])
                else:
                    nc.sync.dma_start(xq[:], x_d[q])
                nc.sync.dma_start(tq[:], t_d[q])

                # s = sigmoid(-x) (+ accum sum s for the quad)
                sq = s_pool.tile([P, QF], bf16, tag="s", name=f"sq{q}")
                nc.scalar.activation(
                    sq[:], xq[:], AFT.Sigmoid, scale=-1.0,
                    accum_out=stats_act[:, 2 * q:2 * q + 1],
                )
                # pred = (x >= 0.5) in bf16 (4x mode)
                pq = pred_pool.tile([P, QF], bf16, tag="pred", name=f"pq{q}")
                nc.vector.tensor_scalar(
                    out=pq[:], in0=xq[:], scalar1=0.5, scalar2=None,
                    op0=AluOpType.is_ge,
                )
                # sum(x*t) for the quad via fused STT accumulate
                uq = scr_pool.tile([P, QF], bf16, tag="u", name=f"uq{q}")
                nc.vector.scalar_tensor_tensor(
                    out=uq[:], in0=xq[:], scalar=1.0, in1=tq[:],
                    op0=AluOpType.mult, op1=AluOpType.mult,
                    accum_out=stats_dve[:, q:q + 1],
                )

                # Per-slab PE reductions.
                for j in range(QS):
                    s_i = q * QS + j
                    base = j * F
                    first = s_i == 0
                    last = s_i == SLABS - 1
                    for c in range(16):
                        sl = slice(base + c * 128, base + (c + 1) * 128)
                        nc.tensor.matmul(p_st[:, :], sq[:, sl], tq[:, sl],
                                         start=(first and c == 0),
                                         stop=(last and c == 15))
                    tp_bank = p_tp[s_i % 2]
                    for c in range(16):
                        sl = slice(base + c * 128, base + (c + 1) * 128)
                        nc.tensor.matmul(tp_bank[:, :], pq[:, sl], tq[:, sl],
                                         start=(c == 0), stop=(c == 15))
                    mtp = scr_pool.tile([P, 128], f32, tag="mtp",
                                        name=f"mtp{s_i}")
                    nc.vector.tensor_tensor(out=mtp[:], in0=tp_bank[:, :],
                                            in1=ident[:], op=AluOpType.mult)
                    nc.gpsimd.dma_start(dtp_d[s_i], mtp[:])

                    row = (s_i % 3) * 32
                    colblk = ((s_i // 3) % 2) * 256
                    t_bank = p_t[s_i // 6]
                    pr_bank = p_pr[s_i // 6]
                    for c in range(8):
                        sl = slice(base + c * 256, base + (c + 1) * 256)
                        nc.tensor.matmul(
                            t_bank[row:row + 1, colblk:colblk + 256],
                            ones[:], tq[:, sl], start=(c == 0), stop=(c == 7))
                    for c in range(8):
                        sl = slice(base + c * 256, base + (c + 1) * 256)
                        nc.tensor.matmul(
                            pr_bank[row:row + 1, colblk:colblk + 256],
                            ones[:], pq[:, sl], start=(c == 0), stop=(c == 7))

                # ln(s) for this quad (+ accum)
                lq = scr_pool.tile([P, QF], bf16, tag="l", bufs=1, name=f"lq{q}")
                nc.scalar.activation(
                    lq[:], sq[:], AFT.Ln,
                    accum_out=stats_act[:, 2 * q + 1:2 * q + 2],
                )

            # ---- Epilogue ----
            mst = misc_pool.tile([P, 128], f32)
            nc.vector.tensor_tensor(out=mst[:], in0=p_st[:, :], in1=ident[:],
                                    op=AluOpType.mult)
            nc.sync.dma_start(dst_d[:], mst[:])

            trows = misc_pool.tile([96, 2048], f32)
            for i in range(2):
                nc.vector.tensor_copy(trows[0:96, 512 * i:512 * (i + 1)],
                                      p_t[i][0:96, :])
                nc.vector.tensor_copy(trows[0:96, 1024 + 512 * i:1024 + 512 * (i + 1)],
                                      p_pr[i][0:96, :])
            nc.sync.dma_start(tr_d[:], trows[:])
            nc.sync.dma_start(sa_d[:], stats_act[:])
            nc.sync.dma_start(sd_d[:], stats_dve[:])

    nc.compile()
    _CACHED["nc"] = nc
    return nc


def _to_bf16_bits(a: np.ndarray) -> np.ndarray:
    """f32 -> bf16 bits with round-to-nearest-even, returned as uint16."""
    u = np.ascontiguousarray(a, dtype=np.float32).view(np.uint32)
    rounded = ((u + 0x7FFF + ((u >> 16) & 1)) >> 16).astype(np.uint16)
    return rounded


def _shard_inputs(logits: np.ndarray, targets: np.ndarray):
    import ml_dtypes

    bf = ml_dtypes.bfloat16
    xb = _to_bf16_bits(logits).view(bf)
    tb = _to_bf16_bits(targets).view(bf)
    eye = np.eye(P, 128, dtype=np.float32).astype(bf)
    in_maps = []
    for i in range(N_CORES):
        sl = slice(i * D_SHARD, (i + 1) * D_SHARD)
        x = np.ascontiguousarray(xb[:, :, sl]).reshape(QUADS, P, QF)
        t = np.ascontiguousarray(tb[:, :, sl]).reshape(QUADS, P, QF)
        in_maps.append({"logits": x, "targets": t, "ident": eye})
    return in_maps


def _combine(results):
    """Host-side reduction of per-core partials to the scalar loss."""
    EPS = 1e-9
    S_tp = np.zeros(SLABS)
    S_t = np.zeros(SLABS)
    S_pred = np.zeros(SLABS)
    S_s = 0.0
    S_l = 0.0
    S_xt = 0.0
    S_st = 0.0
    for r in results:
        sa = r["stats_act"].astype(np.float64)
        S_s += sa[:, 0::2].sum()
        S_l += sa[:, 1::2].sum()
        S_xt += r["stats_dve"].astype(np.float64).sum()
        S_st += r["diag_st"].astype(np.float64).sum()
        tr = r["trows"].astype(np.float64)
        dtp = r["diag_tp"].astype(np.float64)
        for s_i in range(SLABS):
            S_tp[s_i] += dtp[s_i].sum()
            row = (s_i % 3) * 32
            col = 512 * (s_i // 6) + 256 * ((s_i // 3) % 2)
            S_t[s_i] += tr[row, col:col + 256].sum()
            S_pred[s_i] += tr[row, 1024 + col:1024 + col + 256].sum()

    sum_prob = N_TOTAL - S_s
    sum_pt = S_t.sum() - S_st               # sum(prob * t)
    sum_sp = -S_l                           # sum(softplus(x))
    bce = (sum_sp - S_xt) / N_TOTAL

    union = sum_prob + S_t.sum()
    inter = 2.0 * sum_pt
    dice_loss = 1.0 - (inter + EPS) / union

    score = np.where(
        (S_t == 0) & (S_pred == 0),
        np.ones_like(S_t),
        (2.0 * S_tp + EPS) / (S_t + S_pred),
    ).reshape(B, C)
    per_class = score.mean(axis=0)

    loss = (bce + dice_loss * 0.5 + per_class[0] * 0.2
            + per_class[1] * 0.1 + per_class[2] * 0.2)
    return np.float32(loss)


def kernel(logits: np.ndarray, targets: np.ndarray) -> np.ndarray:
    nc = _build()
    in_maps = _shard_inputs(np.asarray(logits), np.asarray(targets))
    res = run_bass_kernel_spmd(nc, in_maps, list(range(N_CORES)))
    return _combine(res.results)
